# revision 1
# baseline (speedup 1.0000x reference)
"""Context-aware attention pooling kernel for Trainium2 (8 NeuronCores).

Reference computation (per batch b):
    e      = tanh(seq @ W1[:256] + ctx @ W1[256:])      # [T, 64]
    logits = e @ W2                                      # [T, 1]
    a      = softmax(logits over T)
    out    = sum_t a[t] * seq[t]                         # [256]

Shapes: B=64, T=4096, D1=256, D2=128, UNITS=64.
Sharding: data-parallel over batch, 8 batches per core; W1/W2 replicated.

Per-core program (all t-tiles are 128 rows):
  - seq[b] loaded in natural layout [t, d] as bf16 (f32->bf16 cast inside the
    SWDGE DMA), tile layout nat[p, n*256 + d] = seq[b, n*128+p, d]
  - pair-transpose: adjacent-d bf16 pairs are reinterpreted as one f32 and
    PE-transposed as f32 blocks (one [128, 128] transpose per t-tile instead
    of two); the e-matmul reads the pair layout with stride-2 bf16 APs
    against even/odd-row-interleaved W1 copies
  - e-matmul contracts d on PE: eT2[u-half, t] (tanh + ctx-bias on ScalarE),
    with even/odd 512-t chunks col-packed into both halves of the PE array
  - logits as PE matmuls into PSUM columns (t lands on partitions), row-packed
    across the two eT2 halves
  - softmax without max-subtraction (|logit| <= ||W2||_1, safe in f32);
    Exp + per-partition sums fused on ScalarE; total Z via a ones-matmul;
    the single 1/Z scale is applied to the pooled output at the end
  - pooling on PE: p-columns stationary (1-col weight loads), natural seq
    tiles moving, accumulated over the 32 t-tiles into PSUM [1, 256]
  - np.eye ships as an input so the gpsimd queue only carries seq descriptors;
    dummy ident matmuls warm the PE clock (HAM) during the initial DMA ramp
"""

import numpy as np

import concourse.bacc as bacc
import concourse.mybir as mybir
from concourse.tile import TileContext

F32 = mybir.dt.float32
BF16 = mybir.dt.bfloat16

N_CORES = 8
B_CORE = 8          # batches per core
T = 4096
D1 = 256
D2 = 128
U = 64
NT = T // 128       # 32 t-tiles per batch


def build_program():
    nc = bacc.Bacc("TRN2", target_bir_lowering=False, debug=False)

    seq = nc.declare_dram_parameter("seq", [B_CORE, T, D1], F32, isOutput=False)
    ctx = nc.declare_dram_parameter("ctx", [B_CORE, D2], F32, isOutput=False)
    w1 = nc.declare_dram_parameter("w1", [D1 + D2, U], F32, isOutput=False)
    w2 = nc.declare_dram_parameter("w2", [U, 1], F32, isOutput=False)
    # identity fed as data (np.eye) so the GpSimd queue never stalls the seq
    # descriptor stream on an affine_select
    ident_in = nc.declare_dram_parameter("ident_in", [128, 128], F32, isOutput=False)
    outp = nc.declare_dram_parameter("outp", [1, B_CORE * D1], F32, isOutput=True)

    with TileContext(nc) as tc:
        with (
            tc.tile_pool(name="singles", bufs=1) as singles,
            tc.tile_pool(name="nat_pool", bufs=4) as nat_pool,
            tc.tile_pool(name="seqt_pool", bufs=2) as seqt_pool,
            tc.tile_pool(name="et_pool", bufs=2) as et_pool,
            tc.tile_pool(name="small_pool", bufs=2) as small_pool,
            tc.tile_pool(name="ps", bufs=1, space="PSUM") as ps,
        ):
            # identity via HWDGE (independent of the gpsimd queue)
            ident = singles.tile([128, 128], F32)
            nc.sync.dma_start(out=ident, in_=ident_in[:, :])
            ident8 = ident[0:8, 0:8]

            # W1[0:256] interleaved as [q, (s u)]: cols 0:64 = even rows
            # (d = 2q), cols 64:128 = odd rows (d = 2q+1); SWDGE handles the
            # 3D access pattern and the f32->bf16 cast. This is the only
            # setup work on the gpsimd queue ahead of the seq loads.
            w1eo = singles.tile([128, 2 * U], BF16)
            nc.gpsimd.dma_start(
                out=w1eo.rearrange("q (s u) -> q s u", s=2),
                in_=w1[0:256].rearrange("(q s) u -> q s u", s=2),
            )

            # HAM warm-up: dense dummy matmuls in the otherwise data-starved
            # ramp window so batch 0 computes at the full 2.4 GHz clock
            warm_ps = ps.tile([128, 128], F32, tag="z", bufs=1)
            for _ in range(40):
                nc.tensor.matmul(warm_ps, lhsT=ident, rhs=ident, start=True, stop=True)

            # ---- seq loads (natural layout, f32 -> bf16 cast in the DMA);
            # each batch is 4 chunks so consumers start on partial data
            nat_tiles = [None] * B_CORE

            def load_nat(b):
                nat = nat_pool.tile(
                    [128, NT * D1], BF16, tag="nat", name=f"nat{b}"
                )
                # t is loaded permuted as t = 256m + 2p + s so each HBM
                # descriptor covers 2 consecutive t rows (2 KiB contiguous,
                # half the descriptor overhead). The softmax+pool pipeline is
                # invariant to any fixed t-permutation as long as nat, the
                # transposes, logits and p-columns share it -- they all index
                # the same tile layout, so nothing else changes.
                seq_b = seq[b].rearrange("(m p s) d -> p m (s d)", p=128, s=2)
                nat_3d = nat.rearrange("p (m sd) -> p m sd", sd=2 * D1)
                for q in range(4):
                    nsl = slice(4 * q, 4 * (q + 1))
                    nc.gpsimd.dma_start(out=nat_3d[:, nsl], in_=seq_b[:, nsl])
                nat_tiles[b] = nat

            load_nat(0)
            load_nat(1)
            load_nat(2)

            w1c = singles.tile([128, U], F32)
            nc.sync.dma_start(out=w1c, in_=w1[256:384, :])

            w2st = singles.tile([128, 1], F32)
            nc.sync.dma_start(out=w2st[0:U], in_=w2[:, :])
            nc.sync.dma_start(out=w2st[U:128], in_=w2[:, :])
            w2t2 = singles.tile([128, 1], BF16)
            nc.vector.tensor_copy(w2t2, w2st)

            ctx_nat = singles.tile([B_CORE, D2], F32)
            nc.sync.dma_start(out=ctx_nat, in_=ctx[:, :])
            ctxT_ps = ps.tile([D2, B_CORE], F32, tag="lgA", bufs=1)
            nc.tensor.transpose(ctxT_ps, ctx_nat, ident8)
            ctxT = singles.tile([D2, B_CORE], F32)
            nc.vector.tensor_copy(ctxT, ctxT_ps)

            # all 8 context projections at once, duplicated on both partition
            # halves (tanh bias for even/odd chunks): cb_all[64h + u, b]
            cb_ps = ps.tile([128, B_CORE], F32, tag="lgB", bufs=1)
            nc.tensor.matmul(cb_ps[0:U], lhsT=w1c, rhs=ctxT, start=True, stop=True)
            nc.tensor.matmul(
                cb_ps[U:128],
                lhsT=w1c,
                rhs=ctxT,
                start=True,
                stop=True,
                tile_position=(0, U),
            )
            cb_all = singles.tile([128, B_CORE], F32)
            nc.scalar.copy(cb_all, cb_ps)

            ones_col = singles.tile([128, 1], F32)
            nc.vector.memset(ones_col, 1.0)

            final_sb = singles.tile([1, B_CORE * D1], F32)

            # ---- per-batch pipeline ----
            for b in range(B_CORE):
                nat = nat_tiles[b]
                if b + 3 < B_CORE:
                    load_nat(b + 3)

                # Pair-transpose trick: reinterpret the bf16 pair
                # (seq[t, 2q], seq[t, 2q+1]) as one f32 and PE-transpose f32
                # blocks -- one [128, 128] transpose per t-tile instead of two.
                # seqTp[q, 2t + s] (bf16 view) = seq[t, 2q + s].
                nat_f32 = nat.bitcast(F32)
                seqTp = seqt_pool.tile([128, T], F32, tag="seqTp", name=f"sTp{b}")
                for k in range(NT // 4):
                    pst = ps.tile([128, 512], F32, tag="tp", bufs=2)
                    for i in range(4):
                        n = 4 * k + i
                        nc.tensor.transpose(
                            pst[:, 128 * i : 128 * (i + 1)],
                            nat_f32[:, 128 * n : 128 * (n + 1)],
                            ident,
                        )
                    nc.vector.tensor_copy(seqTp[:, 512 * k : 512 * (k + 1)], pst)
                # [128, s, t] bf16 view: s=0 -> even d rows, s=1 -> odd
                stp = seqTp.bitcast(BF16).rearrange("p (t s) -> p s t", s=2)

                # e = tanh(z + cb) as eT2 [128, 2048] bf16: even 512-chunks of
                # t on partitions 0..63, odd chunks on partitions 64..127 (so
                # logits matmuls can row-pack into both halves of the PE array)
                eT2 = et_pool.tile([128, T // 2], BF16, tag="eT2", name=f"eT2_{b}")
                for c in range(T // 512):
                    par = c % 2
                    rsl = slice(U * par, U * par + U)
                    e_ps = ps.tile([128, 512], F32, tag="e", bufs=2)
                    sl = slice(512 * c, 512 * (c + 1))
                    tp = (0, U * par)
                    nc.tensor.matmul(
                        e_ps[rsl],
                        lhsT=w1eo[:, 0:U],
                        rhs=stp[:, 0, sl],
                        start=True,
                        stop=False,
                        tile_position=tp,
                    )
                    nc.tensor.matmul(
                        e_ps[rsl],
                        lhsT=w1eo[:, U : 2 * U],
                        rhs=stp[:, 1, sl],
                        start=False,
                        stop=True,
                        tile_position=tp,
                    )
                    nc.scalar.activation(
                        eT2[rsl, 512 * (c // 2) : 512 * (c // 2) + 512],
                        e_ps[rsl],
                        mybir.ActivationFunctionType.Tanh,
                        bias=cb_all[rsl, b : b + 1],
                    )

                # logits in two row-packed streams: tile n -> chunk c = n//4,
                # parity c%2, column j = 4*(c//2) + n%4 of lgA (even) / lgB
                lgA = ps.tile([128, NT // 2], F32, tag="lgA", bufs=1)
                lgB = ps.tile([128, NT // 2], F32, tag="lgB", bufs=1)
                for c2 in range(T // 1024):
                    for i in range(4):
                        j = 4 * c2 + i
                        csl = slice(128 * j, 128 * (j + 1))
                        nc.tensor.matmul(
                            lgA[:, j : j + 1],
                            lhsT=eT2[0:U, csl],
                            rhs=w2t2[0:U],
                            start=True,
                            stop=True,
                        )
                        nc.tensor.matmul(
                            lgB[:, j : j + 1],
                            lhsT=eT2[U:128, csl],
                            rhs=w2t2[U:128],
                            start=True,
                            stop=True,
                        )

                # p = exp(logits) with fused per-partition sums
                pA = small_pool.tile([128, NT // 2], BF16, tag="pA")
                pB = small_pool.tile([128, NT // 2], BF16, tag="pB")
                sumA = small_pool.tile([128, 1], F32, tag="sumA")
                sumB = small_pool.tile([128, 1], F32, tag="sumB")
                nc.scalar.activation(
                    pA, lgA, mybir.ActivationFunctionType.Exp, accum_out=sumA
                )
                nc.scalar.activation(
                    pB, lgB, mybir.ActivationFunctionType.Exp, accum_out=sumB
                )
                psums = small_pool.tile([128, 1], F32, tag="psums")
                nc.vector.tensor_add(psums, sumA, sumB)

                # Z = sum over partitions of psums
                z_ps = ps.tile([1, 1], F32, tag="z", bufs=1)
                nc.tensor.matmul(z_ps, lhsT=psums, rhs=ones_col, start=True, stop=True)
                invz = small_pool.tile([1, 1], F32, tag="invz")
                nc.vector.reciprocal(invz, z_ps)

                # pooling: out[d] = sum_t p[t] * seq[t, d], accumulated on PE
                pool_ps = ps.tile([1, D1], F32, tag="pool", bufs=1)
                for n in range(NT):
                    c = n // 4
                    j = 4 * (c // 2) + n % 4
                    p_col = (pA if c % 2 == 0 else pB)[:, j : j + 1]
                    nc.tensor.matmul(
                        pool_ps,
                        lhsT=p_col,
                        rhs=nat[:, 256 * n : 256 * (n + 1)],
                        start=(n == 0),
                        stop=(n == NT - 1),
                    )

                # normalize by 1/Z while evacuating to SBUF, store per batch
                nc.scalar.activation(
                    final_sb[0:1, D1 * b : D1 * (b + 1)],
                    pool_ps,
                    mybir.ActivationFunctionType.Copy,
                    scale=invz,
                )
                nc.sync.dma_start(
                    out=outp[0:1, D1 * b : D1 * (b + 1)],
                    in_=final_sb[0:1, D1 * b : D1 * (b + 1)],
                )

    nc.compile()
    return nc


_NC_CACHE = []


def _get_program():
    if not _NC_CACHE:
        _NC_CACHE.append(build_program())
    return _NC_CACHE[0]


def make_in_maps(sequence, context, W1, W2):
    ident = np.eye(128, dtype=np.float32)
    in_maps = []
    for c in range(N_CORES):
        sl = slice(B_CORE * c, B_CORE * (c + 1))
        in_maps.append(
            {
                "seq": np.ascontiguousarray(sequence[sl], dtype=np.float32),
                "ctx": np.ascontiguousarray(context[sl], dtype=np.float32),
                "w1": np.ascontiguousarray(W1, dtype=np.float32),
                "w2": np.ascontiguousarray(W2, dtype=np.float32),
                "ident_in": ident,
            }
        )
    return in_maps


def kernel(sequence, context, W1, W2):
    """Full-input entry point: shards batch across 8 cores, returns [64, 256] f32."""
    from concourse.bass_utils import run_bass_kernel_spmd

    nc = _get_program()
    in_maps = make_in_maps(sequence, context, W1, W2)
    res = run_bass_kernel_spmd(nc, in_maps, list(range(N_CORES)))
    out = np.concatenate(
        [res.results[c]["outp"].reshape(B_CORE, D1) for c in range(N_CORES)], axis=0
    )
    return out.astype(np.float32)



# revision 5
# speedup vs baseline: 1.0898x; 1.0898x over previous
"""Context-aware attention pooling kernel for Trainium2 (8 NeuronCores).

Reference computation (per batch b):
    e      = tanh(seq @ W1[:256] + ctx @ W1[256:])      # [T, 64]
    logits = e @ W2                                      # [T, 1]
    a      = softmax(logits over T)
    out    = sum_t a[t] * seq[t]                         # [256]

Shapes: B=64, T=4096, D1=256, D2=128, UNITS=64.
Sharding: data-parallel over batch, 8 batches per core; W1/W2 replicated.

Per-core program (all t-tiles are 128 rows):
  - seq[b] loaded as bf16 (f32->bf16 cast in the SWDGE DMA) in natural layout
    nat[p, n*256 + d]; t is loaded permuted (t = 512m + 4p + s) so each HBM
    descriptor covers 4 consecutive t rows (4 KiB contiguous). The whole
    pipeline is invariant to a fixed t-permutation.
  - seq transposed on PE as plain bf16 [128,128] blocks (1 cycle/row streams,
    FWL-eligible weight loads) into seqT [128, 2T]: partition q = d%128,
    half h = d//128 at column h*T + t.
  - e-matmul: z^T = W1h.T @ seqT_h accumulated over the two d-halves;
    even/odd 512-t chunks col-packed into both PE halves; tanh + ctx bias
    fused on ScalarE into eT2 [128, 2048] bf16 (even chunks rows 0:63).
  - logits: row-packed 1-col matmuls into a single PSUM bank lg [128, 32]
    (even-chunk tiles cols 0:15, odd cols 16:31).
  - softmax without max-subtraction (|logit| <= ||W2||_1, safe in f32); one
    Exp+accum activation for the whole batch; total Z via a ones-matmul;
    the single 1/Z scale applied to the pooled output at the end.
  - pooling on PE, row-split packed: p-columns stationary, nat moving, the
    128 t-rows split into two 64-row tiles at tile_position (0,0)/(64,0)
    accumulating into two PSUM banks (streams overlap), summed at the end.
  - gpsimd (SWDGE) queue carries ONLY seq descriptors; all small loads go via
    HWDGE (sync) + on-chip casts; np.eye ships as input; bf16 dummy matmuls
    warm the PE clock (HAM) during the initial DMA ramp.
"""

import numpy as np

import concourse.bacc as bacc
import concourse.mybir as mybir
from concourse.tile import TileContext

F32 = mybir.dt.float32
BF16 = mybir.dt.bfloat16

N_CORES = 8
B_CORE = 8          # batches per core
T = 4096
D1 = 256
D2 = 128
U = 64
NT = T // 128       # 32 t-tiles per batch


def build_program():
    nc = bacc.Bacc("TRN2", target_bir_lowering=False, debug=False)

    seq = nc.declare_dram_parameter("seq", [B_CORE, T, D1], F32, isOutput=False)
    ctx = nc.declare_dram_parameter("ctx", [B_CORE, D2], F32, isOutput=False)
    w1 = nc.declare_dram_parameter("w1", [D1 + D2, U], F32, isOutput=False)
    w2 = nc.declare_dram_parameter("w2", [U, 1], F32, isOutput=False)
    ident_in = nc.declare_dram_parameter("ident_in", [128, 128], F32, isOutput=False)
    outp = nc.declare_dram_parameter("outp", [1, B_CORE * D1], F32, isOutput=True)

    with TileContext(nc) as tc:
        with (
            tc.tile_pool(name="singles", bufs=1) as singles,
            tc.tile_pool(name="nat_pool", bufs=5) as nat_pool,
            tc.tile_pool(name="seqt_pool", bufs=2) as seqt_pool,
            tc.tile_pool(name="et_pool", bufs=2) as et_pool,
            tc.tile_pool(name="small_pool", bufs=2) as small_pool,
            tc.tile_pool(name="ps", bufs=1, space="PSUM") as ps,
        ):
            # ---- seq loads first: the SWDGE queue must start streaming ASAP
            nat_tiles = [None] * B_CORE

            def load_nat(b):
                nat = nat_pool.tile(
                    [128, NT * D1], BF16, tag="nat", name=f"nat{b}"
                )
                # t permuted as t = 512m + 4p + s: each descriptor covers 4
                # consecutive t rows (4 KiB contiguous in HBM). All consumers
                # share the same tile layout, so the permutation cancels.
                seq_b = seq[b].rearrange("(m p s) d -> p m (s d)", p=128, s=4)
                nat_3d = nat.rearrange("p (m sd) -> p m sd", sd=4 * D1)
                for q in range(4):
                    nsl = slice(2 * q, 2 * (q + 1))
                    nc.gpsimd.dma_start(out=nat_3d[:, nsl], in_=seq_b[:, nsl])
                nat_tiles[b] = nat

            load_nat(0)
            load_nat(1)
            load_nat(2)
            load_nat(3)

            # ---- small setup, all via HWDGE + on-chip casts
            identf = singles.tile([128, 128], F32)
            nc.sync.dma_start(out=identf, in_=ident_in[:, :])
            identb = singles.tile([128, 128], BF16)
            nc.vector.tensor_copy(identb, identf)
            ident8 = identf[0:8, 0:8]

            # W1 sequence half, d-split: w1A = rows 0:128, w1B = rows 128:256
            w1A_f = singles.tile([128, U], F32)
            nc.sync.dma_start(out=w1A_f, in_=w1[0:128, :])
            w1B_f = singles.tile([128, U], F32)
            nc.sync.dma_start(out=w1B_f, in_=w1[128:256, :])
            w1A = singles.tile([128, U], BF16)
            nc.vector.tensor_copy(w1A, w1A_f)
            w1B = singles.tile([128, U], BF16)
            nc.vector.tensor_copy(w1B, w1B_f)

            w1c = singles.tile([128, U], F32)
            nc.sync.dma_start(out=w1c, in_=w1[256:384, :])

            w2st = singles.tile([128, 1], F32)
            nc.sync.dma_start(out=w2st[0:U], in_=w2[:, :])
            nc.sync.dma_start(out=w2st[U:128], in_=w2[:, :])
            w2t2 = singles.tile([128, 1], BF16)
            nc.vector.tensor_copy(w2t2, w2st)

            # HAM warm-up: cheap fp32 dummy matmuls (64-col) during DMA ramp
            warm_ps = ps.tile([128, 128], F32, tag="z", bufs=1)
            for _ in range(40):
                nc.tensor.matmul(
                    warm_ps[0:64, 0:64], lhsT=identf[:, 0:64], rhs=identf[:, 0:64],
                    start=True, stop=True,
                )

            ctx_nat = singles.tile([B_CORE, D2], F32)
            nc.sync.dma_start(out=ctx_nat, in_=ctx[:, :])
            ctxT_ps = ps.tile([D2, B_CORE], F32, tag="e", bufs=2)
            nc.tensor.transpose(ctxT_ps, ctx_nat, ident8)
            ctxT = singles.tile([D2, B_CORE], F32)
            nc.vector.tensor_copy(ctxT, ctxT_ps)

            # all 8 context projections, duplicated on both partition halves
            cb_ps = ps.tile([128, B_CORE], F32, tag="lg", bufs=1)
            nc.tensor.matmul(cb_ps[0:U], lhsT=w1c, rhs=ctxT, start=True, stop=True)
            nc.tensor.matmul(
                cb_ps[U:128], lhsT=w1c, rhs=ctxT, start=True, stop=True,
                tile_position=(0, U),
            )
            cb_all = singles.tile([128, B_CORE], F32)
            nc.scalar.copy(cb_all, cb_ps)

            ones_col = singles.tile([128, 1], F32)
            nc.vector.memset(ones_col, 1.0)

            final_sb = singles.tile([1, B_CORE * D1], F32)

            # ---- per-batch pipeline ----
            for b in range(B_CORE):
                nat = nat_tiles[b]
                if b + 4 < B_CORE:
                    load_nat(b + 4)

                # seqT[q, h*T + t] = seq-val at d = 128h + q, permuted-t t.
                seqT = seqt_pool.tile([128, 2 * T], BF16, tag="seqT", name=f"sT{b}")
                eT2 = et_pool.tile([128, T // 2], BF16, tag="eT2", name=f"eT2_{b}")

                for c in range(8):          # 512-t chunk = t-tiles 4c..4c+3
                    pst = ps.tile([128, 1024], BF16, tag="tp", bufs=2)
                    for h in range(2):
                        for i in range(4):
                            n = 4 * c + i
                            nc.tensor.transpose(
                                pst[:, 512 * h + 128 * i : 512 * h + 128 * (i + 1)],
                                nat[:, 256 * n + 128 * h : 256 * n + 128 * (h + 1)],
                                identb,
                            )
                    for h in range(2):
                        nc.vector.tensor_copy(
                            seqT[:, h * T + 512 * c : h * T + 512 * (c + 1)],
                            pst[:, 512 * h : 512 * (h + 1)],
                        )

                    # z^T for this chunk: contract both d-halves, col-packed
                    par = c % 2
                    rsl = slice(U * par, U * par + U)
                    tp = (0, U * par)
                    e_ps = ps.tile([128, 512], F32, tag="e", bufs=2)
                    sl = slice(512 * c, 512 * (c + 1))
                    nc.tensor.matmul(
                        e_ps[rsl], lhsT=w1A, rhs=seqT[:, sl],
                        start=True, stop=False, tile_position=tp,
                    )
                    nc.tensor.matmul(
                        e_ps[rsl], lhsT=w1B,
                        rhs=seqT[:, T + 512 * c : T + 512 * (c + 1)],
                        start=False, stop=True, tile_position=tp,
                    )
                    nc.scalar.activation(
                        eT2[rsl, 512 * (c // 2) : 512 * (c // 2) + 512],
                        e_ps[rsl],
                        mybir.ActivationFunctionType.Tanh,
                        bias=cb_all[rsl, b : b + 1],
                    )

                # logits: single PSUM bank; even-chunk tiles -> cols 0:15,
                # odd-chunk tiles -> cols 16:31. Tile n -> col
                # 16*(c%2) + 4*(c//2) + n%4 with c = n//4.
                lg = ps.tile([128, 2 * NT // 2], F32, tag="lg", bufs=1)
                for j in range(NT // 2):
                    csl = slice(128 * j, 128 * (j + 1))
                    nc.tensor.matmul(
                        lg[:, j : j + 1],
                        lhsT=eT2[0:U, csl], rhs=w2t2[0:U],
                        start=True, stop=True,
                    )
                    nc.tensor.matmul(
                        lg[:, 16 + j : 17 + j],
                        lhsT=eT2[U:128, csl], rhs=w2t2[U:128],
                        start=True, stop=True,
                    )

                # p = exp(logits), one activation with fused row-sums
                pAB = small_pool.tile([128, 2 * NT // 2], BF16, tag="pAB")
                psums = small_pool.tile([128, 1], F32, tag="psums")
                nc.scalar.activation(
                    pAB, lg, mybir.ActivationFunctionType.Exp, accum_out=psums
                )

                # Z = sum over partitions of psums
                z_ps = ps.tile([1, 1], F32, tag="z", bufs=1)
                nc.tensor.matmul(z_ps, lhsT=psums, rhs=ones_col, start=True, stop=True)
                invz = small_pool.tile([1, 1], F32, tag="invz")
                nc.vector.reciprocal(invz, z_ps)

                # pooling: row-split into two packed 64-row tiles
                poolA = ps.tile([1, D1], F32, tag="poolA", bufs=1)
                poolB = ps.tile([1, D1], F32, tag="poolB", bufs=1)
                for n in range(NT):
                    c = n // 4
                    j16 = 16 * (c % 2) + 4 * (c // 2) + n % 4
                    p_col = pAB[:, j16 : j16 + 1]
                    nsl = slice(256 * n, 256 * (n + 1))
                    nc.tensor.matmul(
                        poolA, lhsT=p_col[0:64], rhs=nat[0:64, nsl],
                        start=(n == 0), stop=(n == NT - 1), tile_position=(0, 0),
                    )
                    nc.tensor.matmul(
                        poolB, lhsT=p_col[64:128], rhs=nat[64:128, nsl],
                        start=(n == 0), stop=(n == NT - 1), tile_position=(64, 0),
                    )

                poolB_sb = small_pool.tile([1, D1], F32, tag="poolB_sb")
                nc.scalar.copy(poolB_sb, poolB)
                pool_sb = small_pool.tile([1, D1], F32, tag="pool_sb")
                nc.vector.tensor_add(pool_sb, poolA, poolB_sb)
                nc.scalar.activation(
                    final_sb[0:1, D1 * b : D1 * (b + 1)],
                    pool_sb,
                    mybir.ActivationFunctionType.Copy,
                    scale=invz,
                )
                nc.sync.dma_start(
                    out=outp[0:1, D1 * b : D1 * (b + 1)],
                    in_=final_sb[0:1, D1 * b : D1 * (b + 1)],
                )

    nc.compile()
    return nc


_NC_CACHE = []


def _get_program():
    if not _NC_CACHE:
        _NC_CACHE.append(build_program())
    return _NC_CACHE[0]


def make_in_maps(sequence, context, W1, W2):
    ident = np.eye(128, dtype=np.float32)
    in_maps = []
    for c in range(N_CORES):
        sl = slice(B_CORE * c, B_CORE * (c + 1))
        in_maps.append(
            {
                "seq": np.ascontiguousarray(sequence[sl], dtype=np.float32),
                "ctx": np.ascontiguousarray(context[sl], dtype=np.float32),
                "w1": np.ascontiguousarray(W1, dtype=np.float32),
                "w2": np.ascontiguousarray(W2, dtype=np.float32),
                "ident_in": ident,
            }
        )
    return in_maps


def kernel(sequence, context, W1, W2):
    """Full-input entry point: shards batch across 8 cores, returns [64, 256] f32."""
    from concourse.bass_utils import run_bass_kernel_spmd

    nc = _get_program()
    in_maps = make_in_maps(sequence, context, W1, W2)
    res = run_bass_kernel_spmd(nc, in_maps, list(range(N_CORES)))
    out = np.concatenate(
        [res.results[c]["outp"].reshape(B_CORE, D1) for c in range(N_CORES)], axis=0
    )
    return out.astype(np.float32)
